# revision 46
# baseline (speedup 1.0000x reference)
"""CrossAttention kernel for 8 Trainium2 NeuronCores.

Data-parallel over batch: core b computes attention for tokens[b].
All device matmuls contract over the partition dim, so tokens are fed
pre-transposed ([hidden, T]) and scores/context vectors are kept in
transposed ([S, T] / [embed, T]) layout until the output projection,
which lands directly in [T, hidden] layout.

Softmax (over S=77) runs in the partition dim: exp on ScalarE (no
max-subtraction needed: scores ~ N(0,1) in f32). The denominator is a
GPSIMD/Pool partition_all_reduce over the (zero-padded to 128 rows)
exp output in SBUF, reciprocal'd on DVE (reciprocal_approx_fast, SBUF
only - custom-DVE PSUM reads are broken on HW); the normalize
multiplies drain attn@V PSUM on DVE. Each head's V columns are
rotated by the head's ctx phase (96h mod 128) so every normalize
piece has read base == write base (14 pieces per chunk).

The bias bo rides for free: V column 80 of head 0 is ones, so the
normalized ctx row 80 is r*denom == 1.0, and the host stores bo in
(otherwise zero-padded) Wo row 80 - no bias add anywhere.

Software pipelining: scores run 2 head-slots ahead and exp 1 slot
ahead of attn@V; chunk c's head loop interleaves the output
projection of chunk c-1 (odd head slots) and the Q projection of
chunk c+1 (head slots 0-5) so the PE never waits on the softmax
chain. Q/out projections share one 3-buf PSUM pool; PSUM = 3+2+3
banks.

Only Exp/Copy run on the scalar engine (one activation-table set: no
LoadActFuncSet thrash). Wq/Wk are zero-padded on the host from
head_dim 80 to 96 so each head's K-slice of Q^T/K^T starts on a
32-aligned partition (PE array row-group granularity; this same
32-alignment rule applies to ALL engines' partition bases).
"""

import numpy as np
import ml_dtypes

import concourse.bass as bass
import concourse.bacc as bacc
import concourse.bass_isa as bass_isa
import concourse.tile as tile
from concourse import mybir
import concourse.bass_utils as bass_utils

F32 = mybir.dt.float32
BF16 = mybir.dt.bfloat16

B, T, S = 8, 4096, 77
HID, EMB, CTX = 640, 640, 768
H, DH = 8, 80
DHP = 96            # head dim padded to a multiple of 32
EMBP = H * DHP      # 768 = 6 partition tiles of 128
KT_H = HID // 128   # 5  k-tiles for hidden-contraction
KT_C = CTX // 128   # 6  k-tiles for ctx-contraction
MT_Q = EMBP // 128  # 6  m-tiles of padded Q^T/K^T rows
TCH = 512           # T chunk (one PSUM bank of f32)
NCH = T // TCH      # 8
P = 128
SCALE = 1.0 / np.sqrt(np.float32(DH))
ES = 1              # exp split factor along the T chunk (attnV is single:
                    # two start=True groups in one PSUM bank re-arm the
                    # bank zero region and clobber the first half)
FP8Q = False        # fp8e4m3 DoubleRow Q projection: 2x PE on the Q GEMM
                    # but measured rel err 3.8e-2 > the 2e-2 gate - off
WQ_SCALE = 32.0     # host scales Wq by this (keeps fp8 out of subnormals);
                    # un-scaled inside the exp's activation scale
F8 = mybir.dt.float8e4
NO1 = 512           # output projection column split (PSUM bank limit)


def _part_cap(base):
    """Max partition count for an engine/PE access starting at `base`
    (within a 128-partition tile): base 0 -> 128, 64 -> 64, 32/96 -> 32."""
    b = base % P
    if b == 0:
        return P
    if b == 64:
        return 64
    assert b % 32 == 0, b
    return 32


def _matmul_segments(row0, nrows):
    """Split rows into (tile, a, b) pieces with legal partition base/count."""
    segs = []
    r = row0
    end = row0 + nrows
    while r < end:
        m, a = r // P, r % P
        c = min(end - r, _part_cap(a), P - a)
        segs.append((m, a, a + c))
        r += c
    return segs


def _dual_segments(write0, read0, nrows):
    """Pieces legal on both a write range starting at write0 (tiled by 128)
    and a read range starting at read0. Yields (m, a, b, r)."""
    off = 0
    while off < nrows:
        w = write0 + off
        r = read0 + off
        m, a = w // P, w % P
        c = min(nrows - off, _part_cap(a), _part_cap(r % P), P - a)
        yield (m, a, a + c, r)
        off += c


def _build_program():
    nc = bacc.Bacc("TRN2", target_bir_lowering=False, debug=False, num_devices=B)

    QDT = F8 if FP8Q else BF16
    tokT = nc.dram_tensor("tokT", [HID, T], QDT, kind="ExternalInput")
    ctxT = nc.dram_tensor("ctxT", [CTX, S], BF16, kind="ExternalInput")
    wqp = nc.dram_tensor("wqp", [HID, EMBP], QDT, kind="ExternalInput")
    wkp = nc.dram_tensor("wkp", [CTX, EMBP], BF16, kind="ExternalInput")
    wv = nc.dram_tensor("wv", [CTX, EMB], BF16, kind="ExternalInput")
    wo = nc.dram_tensor("wo", [EMBP, HID], BF16, kind="ExternalInput")
    out = nc.dram_tensor("out", [T, HID], F32, kind="ExternalOutput")

    tokT_r = tokT.rearrange("(k p) t -> p k t", p=P)

    from contextlib import ExitStack
    with tile.TileContext(nc) as tc, ExitStack() as es:
        consts = es.enter_context(tc.tile_pool(name="consts", bufs=1))
        tok_pool = es.enter_context(tc.tile_pool(name="tok", bufs=3))
        qt_pool = es.enter_context(tc.tile_pool(name="qt", bufs=3))
        attn_pool = es.enter_context(tc.tile_pool(name="attn", bufs=6))
        r_pool = es.enter_context(tc.tile_pool(name="r", bufs=4))
        ctxv_pool = es.enter_context(tc.tile_pool(name="ctxv", bufs=2))
        out_pool = es.enter_context(tc.tile_pool(name="outp", bufs=4))
        # PSUM: qproj/outproj share one 3-buf pool; 3 + 2 + 3 = 8 banks
        ps_qo = es.enter_context(tc.tile_pool(name="ps_qo", bufs=3, space="PSUM"))
        ps_s = es.enter_context(tc.tile_pool(name="ps_s", bufs=2, space="PSUM"))
        ps_cv = es.enter_context(tc.tile_pool(name="ps_cv", bufs=3, space="PSUM"))

        # ---- load weights / context (wq + first token chunks first so the
        # chunk-0 Q projection can start while the rest streams in) ----
        # wq/tok0 split per k-tile so the chunk-0 Q projection's first
        # matmuls start after ~1/5 of the bytes land
        wqp_r = wqp.rearrange("(k p) n -> p k n", p=P)
        wq_sb = consts.tile([P, KT_H, EMBP], QDT)
        toks = {}
        toks[0] = tok_pool.tile([P, KT_H, TCH], QDT, tag="tok", name="tok_sb")
        for k in range(KT_H):
            nc.sync.dma_start(out=wq_sb[:, k, :], in_=wqp_r[:, k, :])
            nc.sync.dma_start(out=toks[0][:, k, :],
                              in_=tokT_r[:, k, 0:TCH])
        wk_sb = consts.tile([P, KT_C, EMBP], BF16)
        nc.sync.dma_start(out=wk_sb, in_=wkp.rearrange("(k p) n -> p k n", p=P))
        ctx_sb = consts.tile([P, KT_C, S], BF16)
        nc.sync.dma_start(out=ctx_sb, in_=ctxT.rearrange("(k p) s -> p k s", p=P))
        wv_sb = consts.tile([P, KT_C, EMB], BF16)
        nc.sync.dma_start(out=wv_sb, in_=wv.rearrange("(k p) n -> p k n", p=P))
        toks[1] = tok_pool.tile([P, KT_H, TCH], QDT, tag="tok", name="tok_sb")
        nc.sync.dma_start(out=toks[1], in_=tokT_r[:, :, TCH:2 * TCH])
        wo_sb = consts.tile([P, MT_Q, HID], BF16)
        nc.sync.dma_start(out=wo_sb, in_=wo.rearrange("(k p) n -> p k n", p=P))

        # ---- Q projection m-tile: 5 PE matmuls + PSUM->SBUF copy on Act ----
        qts = {}

        def emit_qproj_mtile(c, m):
            if m == 0:
                qts[c] = qt_pool.tile([P, MT_Q, TCH], BF16, tag="qt",
                                      name="qt_sb")
            ps_q = ps_qo.tile([P, TCH], F32, tag="ps_qo", name="ps_q")
            mc = slice(m * P, (m + 1) * P)
            if FP8Q:
                # fp8 DoubleRow: each matmul contracts a PAIR of k-tiles
                # (2x PE throughput); the odd 5th k-tile runs plain fp8
                nc.tensor.matmul(
                    ps_q, wq_sb[:, 0:2, mc], toks[c][:, 0:2, :],
                    start=True, stop=False,
                    perf_mode=mybir.MatmulPerfMode.DoubleRow)
                nc.tensor.matmul(
                    ps_q, wq_sb[:, 2:4, mc], toks[c][:, 2:4, :],
                    start=False, stop=False,
                    perf_mode=mybir.MatmulPerfMode.DoubleRow)
                nc.tensor.matmul(
                    ps_q, wq_sb[:, 4, mc], toks[c][:, 4, :],
                    start=False, stop=True)
            else:
                for k in range(KT_H):
                    nc.tensor.matmul(
                        ps_q, wq_sb[:, k, mc],
                        toks[c][:, k, :], start=(k == 0), stop=(k == KT_H - 1))
            nc.scalar.copy(qts[c][:, m, :], ps_q)

        # ---- output projection subtile: 12 PE matmuls, PSUM->SBUF drains
        # split across Act/DVE (no bias add: bo rides Wo row 80), 1 DMA ----
        def emit_outproj_st(ctx_v, c, st):
            tok_cols = ctx_v[:, :, st * P:(st + 1) * P]
            po1 = ps_qo.tile([P, NO1], F32, tag="ps_qo", name="po1")
            for k in range(MT_Q):
                nc.tensor.matmul(po1, tok_cols[:, k, :], wo_sb[:, k, 0:NO1],
                                 start=(k == 0), stop=(k == MT_Q - 1))
            po2 = ps_qo.tile([P, HID - NO1], F32, tag="ps_qo", name="po2")
            for k in range(MT_Q):
                nc.tensor.matmul(po2, tok_cols[:, k, :], wo_sb[:, k, NO1:HID],
                                 start=(k == 0), stop=(k == MT_Q - 1))
            out_sb = out_pool.tile([P, HID], F32)
            nc.scalar.copy(out_sb[:, 0:NO1], po1)
            nc.scalar.copy(out_sb[:, NO1:HID], po2)
            t0 = c * TCH + st * P
            nc.sync.dma_start(out=out[t0:t0 + P, :], in_=out_sb)

        # ---- K^T [EMBP, S] as [128, 6, S] (padded-head rows) ----
        # Emitted after the chunk-0 Q projection in PE order; only needs
        # wk/ctx which stream in behind wq/tok0.
        def emit_kt():
            kt = consts.tile([P, MT_Q, S], BF16, name="kt_sb")
            for m in range(MT_Q):
                ps_k = ps_s.tile([P, S], F32, tag="ps_s", name="ps_k")
                for k in range(KT_C):
                    nc.tensor.matmul(
                        ps_k, wk_sb[:, k, m * P:(m + 1) * P], ctx_sb[:, k, :],
                        start=(k == 0), stop=(k == KT_C - 1))
                nc.vector.tensor_copy(kt[:, m, :], ps_k)
            return kt

        # ---- V [S, H, 128], with each head's columns ROTATED by the
        # head's ctx_v phase phi_h = (96*h) % 128: V dim d sits at column
        # (phi_h + d) % 128. The attn@V output row (96h+off) % 128 then
        # equals ctx_v row 96h+off, so every normalize multiply has
        # read base == write base (14 pieces/chunk instead of 18).
        # Head 0's col 80 is ones: its normalized ctx row is
        # r*denom == 1.0, which multiplies Wo row 80 = bo (host-folded
        # bias). Softmax denominators come from a Pool partition
        # all-reduce over at_sb instead (off the PSUM critical chain).
        def emit_v():
            v = consts.tile([S, H, P], BF16, name="v_sb")
            nc.vector.memset(v, 0.0)
            nc.vector.memset(v[:, 0, DH:DH + 1], 1.0)
            for h in range(H):
                phi = (DHP * h) % P
                ps_v = ps_s.tile([S, DH], F32, tag="ps_s", name="ps_v")
                for k in range(KT_C):
                    nc.tensor.matmul(
                        ps_v, ctx_sb[:, k, :], wv_sb[:, k, h * DH:(h + 1) * DH],
                        start=(k == 0), stop=(k == KT_C - 1))
                d1 = min(P - phi, DH)
                nc.vector.tensor_copy(v[:, h, phi:phi + d1], ps_v[:, 0:d1])
                if d1 < DH:
                    nc.vector.tensor_copy(v[:, h, 0:DH - d1], ps_v[:, d1:DH])
            return v

        def emit_scores(c, h):
            segs = _matmul_segments(h * DHP, DH)
            ps_sc = ps_s.tile([S, TCH], F32, tag="ps_s", name="ps_sc")
            for i, (m, a, b) in enumerate(segs):
                nc.tensor.matmul(
                    ps_sc, kt_sb[a:b, m, :], qts[c][a:b, m, :],
                    start=(i == 0), stop=(i == len(segs) - 1),
                    tile_position=(a, 0))
            return ps_sc

        # ---- chunk-0 prologue ----
        for m in range(MT_Q):
            emit_qproj_mtile(0, m)
        kt_sb = emit_kt()
        v_sb = emit_v()

        prev_ctx = None  # ctx_v of the previous chunk (outproj deferred)
        eng_i = 0
        W = TCH // ES
        for c in range(NCH):
            if c + 2 < NCH:
                toks[c + 2] = tok_pool.tile([P, KT_H, TCH], QDT, tag="tok",
                                            name="tok_sb")
                nc.sync.dma_start(
                    out=toks[c + 2],
                    in_=tokT_r[:, :, (c + 2) * TCH:(c + 3) * TCH])

            def emit_exp(c, h, ps_sc):
                # at padded to 128 partitions (zeros on 77:128) so the
                # Pool all-reduce over all partitions yields the softmax
                # denominator on every row
                at_sb = attn_pool.tile([P, TCH], BF16)
                # zero rows 64:128 (32-aligned base), exp then overwrites
                # 64:77 - leaves 77:128 zero for the partition all-reduce
                nc.gpsimd.memset(at_sb[64:P, :], 0.0)
                escale = SCALE / WQ_SCALE if FP8Q else SCALE
                for e in range(ES):
                    nc.scalar.activation(
                        at_sb[0:S, e * W:(e + 1) * W],
                        ps_sc[:, e * W:(e + 1) * W],
                        mybir.ActivationFunctionType.Exp, scale=float(escale))
                return at_sb

            def emit_recip(at_sb):
                # denominator via Pool all-reduce (SBUF only), reciprocal
                # on DVE from SBUF (custom-DVE PSUM reads are broken on HW)
                ar_sb = r_pool.tile([P, TCH], F32, tag="ar", name="ar_sb")
                nc.gpsimd.partition_all_reduce(
                    ar_sb, at_sb, channels=P, reduce_op=bass_isa.ReduceOp.add)
                rb_sb = r_pool.tile([P, TCH], F32, tag="rb", name="rb_sb")
                nc.vector.reciprocal_approx_fast(out=rb_sb, in_=ar_sb)
                return rb_sb

            # scores run 2 slots ahead, exp 1 slot ahead of attn@V so
            # neither the PE nor the Act queue ever blocks the chain
            def emit_exp_recip(c, h, ps_sc):
                at_sb = emit_exp(c, h, ps_sc)
                return at_sb, emit_recip(at_sb)

            score_ps = {0: emit_scores(c, 0), 1: emit_scores(c, 1)}
            ats = {0: emit_exp_recip(c, 0, score_ps.pop(0))}
            ctx_v = ctxv_pool.tile([P, MT_Q, TCH], BF16)
            for h in range(H):
                if h + 2 < H:
                    score_ps[h + 2] = emit_scores(c, h + 2)
                if h + 1 < H:
                    ats[h + 1] = emit_exp_recip(c, h + 1, score_ps.pop(h + 1))

                # interleaved deferred outproj + next-chunk Q projection:
                # keeps the PE busy while the normalize chain drains ps_cv
                if h % 2 == 1 and prev_ctx is not None:
                    emit_outproj_st(prev_ctx, c - 1, (h - 1) // 2)
                if h < MT_Q and c + 1 < NCH:
                    emit_qproj_mtile(c + 1, h)

                # ctx_aug^T [128, TCH] in the head's rotated row phase
                at_sb, rb_sb = ats.pop(h)
                ps_c = ps_cv.tile([P, TCH], F32, tag="ps_cv", name="ps_c")
                nc.tensor.matmul(ps_c, v_sb[:, h, :], at_sb[0:S, :],
                                 start=True, stop=True)

                # normalized ctx^T into stacked padded [EMBP, TCH] layout;
                # all on DVE (GPSIMD/Pool cannot access PSUM on TRN2);
                # read base == write base thanks to the V rotation;
                # rb was computed off-chain when exp(h) ran
                for (m, a, b) in _matmul_segments(h * DHP, DHP):
                    nc.vector.tensor_mul(
                        ctx_v[a:b, m, :], ps_c[a:b, :], rb_sb[a:b, :])

            prev_ctx = ctx_v

        for st in range(TCH // P):
            emit_outproj_st(prev_ctx, NCH - 1, st)

    nc.compile()
    return nc


_PROGRAM = None


def _get_program():
    global _PROGRAM
    if _PROGRAM is None:
        _PROGRAM = _build_program()
    return _PROGRAM


BF16_NP = ml_dtypes.bfloat16


def _pad_heads(w, dtype=np.float32):
    """[rows, H*DH] -> [rows, H*DHP] zero-padded per head."""
    rows = w.shape[0]
    wp = np.zeros((rows, EMBP), dtype)
    for h in range(H):
        wp[:, h * DHP:h * DHP + DH] = w[:, h * DH:(h + 1) * DH]
    return wp


def _pad_head_rows(w, dtype=np.float32):
    """[H*DH, cols] -> [H*DHP, cols] zero-padded per head."""
    wp = np.zeros((EMBP, w.shape[1]), dtype)
    for h in range(H):
        wp[h * DHP:h * DHP + DH] = w[h * DH:(h + 1) * DH]
    return wp


F8_NP = ml_dtypes.float8_e4m3


def _prepare_in_maps(tokens, context, Wq, Wk, Wv, Wo, bo):
    tokens = np.asarray(tokens, np.float32)
    context = np.asarray(context, np.float32)
    qdt = F8_NP if FP8Q else BF16_NP
    wq_host = _pad_heads(np.asarray(Wq, np.float32))
    if FP8Q:
        wq_host = wq_host * WQ_SCALE
    wqp = wq_host.astype(qdt)
    wkp = _pad_heads(np.asarray(Wk, np.float32)).astype(BF16_NP)
    wv_ = np.ascontiguousarray(np.asarray(Wv, np.float32)).astype(BF16_NP)
    wo_ = _pad_head_rows(np.asarray(Wo, np.float32))
    wo_[DH] = np.asarray(bo, np.float32)   # bias rides Wo pad row 80
    wo_ = wo_.astype(BF16_NP)
    in_maps = []
    for b in range(B):
        in_maps.append({
            "tokT": np.ascontiguousarray(tokens[b].T).astype(qdt),
            "ctxT": np.ascontiguousarray(context[b].T).astype(BF16_NP),
            "wqp": wqp, "wkp": wkp, "wv": wv_, "wo": wo_,
        })
    return in_maps


def kernel(tokens, context, Wq, Wk, Wv, Wo, bo):
    nc = _get_program()
    in_maps = _prepare_in_maps(tokens, context, Wq, Wk, Wv, Wo, bo)
    res = bass_utils.run_bass_kernel_spmd(nc, in_maps, core_ids=list(range(B)))
    return np.stack([res.results[b]["out"] for b in range(B)])


# revision 47
# speedup vs baseline: 1.0005x; 1.0005x over previous
"""CrossAttention kernel for 8 Trainium2 NeuronCores.

Data-parallel over batch: core b computes attention for tokens[b].
All device matmuls contract over the partition dim, so tokens are fed
pre-transposed ([hidden, T]) and scores/context vectors are kept in
transposed ([S, T] / [embed, T]) layout until the output projection,
which lands directly in [T, hidden] layout.

Softmax (over S=77) runs in the partition dim: exp on ScalarE (no
max-subtraction needed: scores ~ N(0,1) in f32). The denominator is a
GPSIMD/Pool partition_all_reduce over the (zero-padded to 128 rows)
exp output in SBUF, reciprocal'd on DVE (reciprocal_approx_fast, SBUF
only - custom-DVE PSUM reads are broken on HW); the normalize
multiplies drain attn@V PSUM on DVE. Each head's V columns are
rotated by the head's ctx phase (96h mod 128) so every normalize
piece has read base == write base (14 pieces per chunk).

The bias bo rides for free: V column 80 of head 0 is ones, so the
normalized ctx row 80 is r*denom == 1.0, and the host stores bo in
(otherwise zero-padded) Wo row 80 - no bias add anywhere.

Software pipelining: scores run 2 head-slots ahead and exp 1 slot
ahead of attn@V; chunk c's head loop interleaves the output
projection of chunk c-1 (odd head slots) and the Q projection of
chunk c+1 (head slots 0-5) so the PE never waits on the softmax
chain. Q/out projections share one 3-buf PSUM pool; PSUM = 3+2+3
banks.

Only Exp/Copy run on the scalar engine (one activation-table set: no
LoadActFuncSet thrash). Wq/Wk are zero-padded on the host from
head_dim 80 to 96 so each head's K-slice of Q^T/K^T starts on a
32-aligned partition (PE array row-group granularity; this same
32-alignment rule applies to ALL engines' partition bases).
"""

import numpy as np
import ml_dtypes

import concourse.bass as bass
import concourse.bacc as bacc
import concourse.bass_isa as bass_isa
import concourse.tile as tile
from concourse import mybir
import concourse.bass_utils as bass_utils

F32 = mybir.dt.float32
BF16 = mybir.dt.bfloat16

B, T, S = 8, 4096, 77
HID, EMB, CTX = 640, 640, 768
H, DH = 8, 80
DHP = 96            # head dim padded to a multiple of 32
EMBP = H * DHP      # 768 = 6 partition tiles of 128
KT_H = HID // 128   # 5  k-tiles for hidden-contraction
KT_C = CTX // 128   # 6  k-tiles for ctx-contraction
MT_Q = EMBP // 128  # 6  m-tiles of padded Q^T/K^T rows
TCH = 512           # T chunk (one PSUM bank of f32)
NCH = T // TCH      # 8
P = 128
SCALE = 1.0 / np.sqrt(np.float32(DH))
ES = 1              # exp split factor along the T chunk (attnV is single:
                    # two start=True groups in one PSUM bank re-arm the
                    # bank zero region and clobber the first half)
FP8Q = False        # fp8e4m3 DoubleRow Q projection: 2x PE on the Q GEMM
                    # but measured rel err 3.8e-2 > the 2e-2 gate - off
WQ_SCALE = 32.0     # host scales Wq by this (keeps fp8 out of subnormals);
                    # un-scaled inside the exp's activation scale
F8 = mybir.dt.float8e4
NO1 = 512           # output projection column split (PSUM bank limit)


def _part_cap(base):
    """Max partition count for an engine/PE access starting at `base`
    (within a 128-partition tile): base 0 -> 128, 64 -> 64, 32/96 -> 32."""
    b = base % P
    if b == 0:
        return P
    if b == 64:
        return 64
    assert b % 32 == 0, b
    return 32


def _matmul_segments(row0, nrows):
    """Split rows into (tile, a, b) pieces with legal partition base/count."""
    segs = []
    r = row0
    end = row0 + nrows
    while r < end:
        m, a = r // P, r % P
        c = min(end - r, _part_cap(a), P - a)
        segs.append((m, a, a + c))
        r += c
    return segs


def _dual_segments(write0, read0, nrows):
    """Pieces legal on both a write range starting at write0 (tiled by 128)
    and a read range starting at read0. Yields (m, a, b, r)."""
    off = 0
    while off < nrows:
        w = write0 + off
        r = read0 + off
        m, a = w // P, w % P
        c = min(nrows - off, _part_cap(a), _part_cap(r % P), P - a)
        yield (m, a, a + c, r)
        off += c


def _build_program():
    nc = bacc.Bacc("TRN2", target_bir_lowering=False, debug=False, num_devices=B)

    QDT = F8 if FP8Q else BF16
    tokT = nc.dram_tensor("tokT", [HID, T], QDT, kind="ExternalInput")
    ctxT = nc.dram_tensor("ctxT", [CTX, S], BF16, kind="ExternalInput")
    wqp = nc.dram_tensor("wqp", [HID, EMBP], QDT, kind="ExternalInput")
    wkp = nc.dram_tensor("wkp", [CTX, EMBP], BF16, kind="ExternalInput")
    wv = nc.dram_tensor("wv", [CTX, EMB], BF16, kind="ExternalInput")
    wo = nc.dram_tensor("wo", [EMBP, HID], BF16, kind="ExternalInput")
    out = nc.dram_tensor("out", [T, HID], F32, kind="ExternalOutput")

    tokT_r = tokT.rearrange("(k p) t -> p k t", p=P)

    from contextlib import ExitStack
    with tile.TileContext(nc) as tc, ExitStack() as es:
        consts = es.enter_context(tc.tile_pool(name="consts", bufs=1))
        tok_pool = es.enter_context(tc.tile_pool(name="tok", bufs=3))
        qt_pool = es.enter_context(tc.tile_pool(name="qt", bufs=3))
        attn_pool = es.enter_context(tc.tile_pool(name="attn", bufs=6))
        r_pool = es.enter_context(tc.tile_pool(name="r", bufs=4))
        ctxv_pool = es.enter_context(tc.tile_pool(name="ctxv", bufs=2))
        out_pool = es.enter_context(tc.tile_pool(name="outp", bufs=4))
        # PSUM: qproj/outproj share one 3-buf pool; 3 + 2 + 3 = 8 banks
        ps_qo = es.enter_context(tc.tile_pool(name="ps_qo", bufs=3, space="PSUM"))
        ps_s = es.enter_context(tc.tile_pool(name="ps_s", bufs=2, space="PSUM"))
        ps_cv = es.enter_context(tc.tile_pool(name="ps_cv", bufs=3, space="PSUM"))

        # ---- load weights / context (wq + first token chunks first so the
        # chunk-0 Q projection can start while the rest streams in) ----
        # wq/tok0 split per k-tile so the chunk-0 Q projection's first
        # matmuls start after ~1/5 of the bytes land
        wqp_r = wqp.rearrange("(k p) n -> p k n", p=P)
        wq_sb = consts.tile([P, KT_H, EMBP], QDT)
        toks = {}
        toks[0] = tok_pool.tile([P, KT_H, TCH], QDT, tag="tok", name="tok_sb")
        for k in range(KT_H):
            nc.sync.dma_start(out=wq_sb[:, k, :], in_=wqp_r[:, k, :])
            # tok0 on the scalar engine's DMA queue: runs in parallel
            # with the wq stream on sync's queue
            nc.scalar.dma_start(out=toks[0][:, k, :],
                                in_=tokT_r[:, k, 0:TCH])
        wk_sb = consts.tile([P, KT_C, EMBP], BF16)
        nc.sync.dma_start(out=wk_sb, in_=wkp.rearrange("(k p) n -> p k n", p=P))
        ctx_sb = consts.tile([P, KT_C, S], BF16)
        nc.sync.dma_start(out=ctx_sb, in_=ctxT.rearrange("(k p) s -> p k s", p=P))
        wv_sb = consts.tile([P, KT_C, EMB], BF16)
        nc.sync.dma_start(out=wv_sb, in_=wv.rearrange("(k p) n -> p k n", p=P))
        toks[1] = tok_pool.tile([P, KT_H, TCH], QDT, tag="tok", name="tok_sb")
        nc.sync.dma_start(out=toks[1], in_=tokT_r[:, :, TCH:2 * TCH])
        wo_sb = consts.tile([P, MT_Q, HID], BF16)
        nc.sync.dma_start(out=wo_sb, in_=wo.rearrange("(k p) n -> p k n", p=P))

        # ---- Q projection m-tile: 5 PE matmuls + PSUM->SBUF copy on Act ----
        qts = {}

        def emit_qproj_mtile(c, m):
            if m == 0:
                qts[c] = qt_pool.tile([P, MT_Q, TCH], BF16, tag="qt",
                                      name="qt_sb")
            ps_q = ps_qo.tile([P, TCH], F32, tag="ps_qo", name="ps_q")
            mc = slice(m * P, (m + 1) * P)
            if FP8Q:
                # fp8 DoubleRow: each matmul contracts a PAIR of k-tiles
                # (2x PE throughput); the odd 5th k-tile runs plain fp8
                nc.tensor.matmul(
                    ps_q, wq_sb[:, 0:2, mc], toks[c][:, 0:2, :],
                    start=True, stop=False,
                    perf_mode=mybir.MatmulPerfMode.DoubleRow)
                nc.tensor.matmul(
                    ps_q, wq_sb[:, 2:4, mc], toks[c][:, 2:4, :],
                    start=False, stop=False,
                    perf_mode=mybir.MatmulPerfMode.DoubleRow)
                nc.tensor.matmul(
                    ps_q, wq_sb[:, 4, mc], toks[c][:, 4, :],
                    start=False, stop=True)
            else:
                for k in range(KT_H):
                    nc.tensor.matmul(
                        ps_q, wq_sb[:, k, mc],
                        toks[c][:, k, :], start=(k == 0), stop=(k == KT_H - 1))
            nc.scalar.copy(qts[c][:, m, :], ps_q)

        # ---- output projection subtile: 12 PE matmuls, PSUM->SBUF drains
        # split across Act/DVE (no bias add: bo rides Wo row 80), 1 DMA ----
        def emit_outproj_st(ctx_v, c, st):
            tok_cols = ctx_v[:, :, st * P:(st + 1) * P]
            po1 = ps_qo.tile([P, NO1], F32, tag="ps_qo", name="po1")
            for k in range(MT_Q):
                nc.tensor.matmul(po1, tok_cols[:, k, :], wo_sb[:, k, 0:NO1],
                                 start=(k == 0), stop=(k == MT_Q - 1))
            po2 = ps_qo.tile([P, HID - NO1], F32, tag="ps_qo", name="po2")
            for k in range(MT_Q):
                nc.tensor.matmul(po2, tok_cols[:, k, :], wo_sb[:, k, NO1:HID],
                                 start=(k == 0), stop=(k == MT_Q - 1))
            out_sb = out_pool.tile([P, HID], F32)
            nc.scalar.copy(out_sb[:, 0:NO1], po1)
            nc.scalar.copy(out_sb[:, NO1:HID], po2)
            t0 = c * TCH + st * P
            nc.sync.dma_start(out=out[t0:t0 + P, :], in_=out_sb)

        # ---- K^T [EMBP, S] as [128, 6, S] (padded-head rows) ----
        # Emitted after the chunk-0 Q projection in PE order; only needs
        # wk/ctx which stream in behind wq/tok0.
        def emit_kt():
            kt = consts.tile([P, MT_Q, S], BF16, name="kt_sb")
            for m in range(MT_Q):
                ps_k = ps_s.tile([P, S], F32, tag="ps_s", name="ps_k")
                for k in range(KT_C):
                    nc.tensor.matmul(
                        ps_k, wk_sb[:, k, m * P:(m + 1) * P], ctx_sb[:, k, :],
                        start=(k == 0), stop=(k == KT_C - 1))
                nc.vector.tensor_copy(kt[:, m, :], ps_k)
            return kt

        # ---- V [S, H, 128], with each head's columns ROTATED by the
        # head's ctx_v phase phi_h = (96*h) % 128: V dim d sits at column
        # (phi_h + d) % 128. The attn@V output row (96h+off) % 128 then
        # equals ctx_v row 96h+off, so every normalize multiply has
        # read base == write base (14 pieces/chunk instead of 18).
        # Head 0's col 80 is ones: its normalized ctx row is
        # r*denom == 1.0, which multiplies Wo row 80 = bo (host-folded
        # bias). Softmax denominators come from a Pool partition
        # all-reduce over at_sb instead (off the PSUM critical chain).
        def emit_v():
            v = consts.tile([S, H, P], BF16, name="v_sb")
            nc.vector.memset(v, 0.0)
            nc.vector.memset(v[:, 0, DH:DH + 1], 1.0)
            for h in range(H):
                phi = (DHP * h) % P
                ps_v = ps_s.tile([S, DH], F32, tag="ps_s", name="ps_v")
                for k in range(KT_C):
                    nc.tensor.matmul(
                        ps_v, ctx_sb[:, k, :], wv_sb[:, k, h * DH:(h + 1) * DH],
                        start=(k == 0), stop=(k == KT_C - 1))
                d1 = min(P - phi, DH)
                nc.vector.tensor_copy(v[:, h, phi:phi + d1], ps_v[:, 0:d1])
                if d1 < DH:
                    nc.vector.tensor_copy(v[:, h, 0:DH - d1], ps_v[:, d1:DH])
            return v

        def emit_scores(c, h):
            segs = _matmul_segments(h * DHP, DH)
            ps_sc = ps_s.tile([S, TCH], F32, tag="ps_s", name="ps_sc")
            for i, (m, a, b) in enumerate(segs):
                nc.tensor.matmul(
                    ps_sc, kt_sb[a:b, m, :], qts[c][a:b, m, :],
                    start=(i == 0), stop=(i == len(segs) - 1),
                    tile_position=(a, 0))
            return ps_sc

        # ---- chunk-0 prologue ----
        for m in range(MT_Q):
            emit_qproj_mtile(0, m)
        kt_sb = emit_kt()
        v_sb = emit_v()

        prev_ctx = None  # ctx_v of the previous chunk (outproj deferred)
        eng_i = 0
        W = TCH // ES
        for c in range(NCH):
            if c + 2 < NCH:
                toks[c + 2] = tok_pool.tile([P, KT_H, TCH], QDT, tag="tok",
                                            name="tok_sb")
                nc.sync.dma_start(
                    out=toks[c + 2],
                    in_=tokT_r[:, :, (c + 2) * TCH:(c + 3) * TCH])

            def emit_exp(c, h, ps_sc):
                # at padded to 128 partitions (zeros on 77:128) so the
                # Pool all-reduce over all partitions yields the softmax
                # denominator on every row
                at_sb = attn_pool.tile([P, TCH], BF16)
                # zero rows 64:128 (32-aligned base), exp then overwrites
                # 64:77 - leaves 77:128 zero for the partition all-reduce
                nc.gpsimd.memset(at_sb[64:P, :], 0.0)
                escale = SCALE / WQ_SCALE if FP8Q else SCALE
                for e in range(ES):
                    nc.scalar.activation(
                        at_sb[0:S, e * W:(e + 1) * W],
                        ps_sc[:, e * W:(e + 1) * W],
                        mybir.ActivationFunctionType.Exp, scale=float(escale))
                return at_sb

            def emit_recip(at_sb):
                # denominator via Pool all-reduce (SBUF only), reciprocal
                # on DVE from SBUF (custom-DVE PSUM reads are broken on HW)
                ar_sb = r_pool.tile([P, TCH], F32, tag="ar", name="ar_sb")
                nc.gpsimd.partition_all_reduce(
                    ar_sb, at_sb, channels=P, reduce_op=bass_isa.ReduceOp.add)
                rb_sb = r_pool.tile([P, TCH], F32, tag="rb", name="rb_sb")
                nc.vector.reciprocal_approx_fast(out=rb_sb, in_=ar_sb)
                return rb_sb

            # scores run 2 slots ahead, exp 1 slot ahead of attn@V so
            # neither the PE nor the Act queue ever blocks the chain
            def emit_exp_recip(c, h, ps_sc):
                at_sb = emit_exp(c, h, ps_sc)
                return at_sb, emit_recip(at_sb)

            score_ps = {0: emit_scores(c, 0), 1: emit_scores(c, 1)}
            ats = {0: emit_exp_recip(c, 0, score_ps.pop(0))}
            ctx_v = ctxv_pool.tile([P, MT_Q, TCH], BF16)
            for h in range(H):
                if h + 2 < H:
                    score_ps[h + 2] = emit_scores(c, h + 2)
                if h + 1 < H:
                    ats[h + 1] = emit_exp_recip(c, h + 1, score_ps.pop(h + 1))

                # interleaved deferred outproj + next-chunk Q projection:
                # keeps the PE busy while the normalize chain drains ps_cv
                if h % 2 == 1 and prev_ctx is not None:
                    emit_outproj_st(prev_ctx, c - 1, (h - 1) // 2)
                if h < MT_Q and c + 1 < NCH:
                    emit_qproj_mtile(c + 1, h)

                # ctx_aug^T [128, TCH] in the head's rotated row phase
                at_sb, rb_sb = ats.pop(h)
                ps_c = ps_cv.tile([P, TCH], F32, tag="ps_cv", name="ps_c")
                nc.tensor.matmul(ps_c, v_sb[:, h, :], at_sb[0:S, :],
                                 start=True, stop=True)

                # normalized ctx^T into stacked padded [EMBP, TCH] layout;
                # all on DVE (GPSIMD/Pool cannot access PSUM on TRN2);
                # read base == write base thanks to the V rotation;
                # rb was computed off-chain when exp(h) ran
                for (m, a, b) in _matmul_segments(h * DHP, DHP):
                    nc.vector.tensor_mul(
                        ctx_v[a:b, m, :], ps_c[a:b, :], rb_sb[a:b, :])

            prev_ctx = ctx_v

        for st in range(TCH // P):
            emit_outproj_st(prev_ctx, NCH - 1, st)

    nc.compile()
    return nc


_PROGRAM = None


def _get_program():
    global _PROGRAM
    if _PROGRAM is None:
        _PROGRAM = _build_program()
    return _PROGRAM


BF16_NP = ml_dtypes.bfloat16


def _pad_heads(w, dtype=np.float32):
    """[rows, H*DH] -> [rows, H*DHP] zero-padded per head."""
    rows = w.shape[0]
    wp = np.zeros((rows, EMBP), dtype)
    for h in range(H):
        wp[:, h * DHP:h * DHP + DH] = w[:, h * DH:(h + 1) * DH]
    return wp


def _pad_head_rows(w, dtype=np.float32):
    """[H*DH, cols] -> [H*DHP, cols] zero-padded per head."""
    wp = np.zeros((EMBP, w.shape[1]), dtype)
    for h in range(H):
        wp[h * DHP:h * DHP + DH] = w[h * DH:(h + 1) * DH]
    return wp


F8_NP = ml_dtypes.float8_e4m3


def _prepare_in_maps(tokens, context, Wq, Wk, Wv, Wo, bo):
    tokens = np.asarray(tokens, np.float32)
    context = np.asarray(context, np.float32)
    qdt = F8_NP if FP8Q else BF16_NP
    wq_host = _pad_heads(np.asarray(Wq, np.float32))
    if FP8Q:
        wq_host = wq_host * WQ_SCALE
    wqp = wq_host.astype(qdt)
    wkp = _pad_heads(np.asarray(Wk, np.float32)).astype(BF16_NP)
    wv_ = np.ascontiguousarray(np.asarray(Wv, np.float32)).astype(BF16_NP)
    wo_ = _pad_head_rows(np.asarray(Wo, np.float32))
    wo_[DH] = np.asarray(bo, np.float32)   # bias rides Wo pad row 80
    wo_ = wo_.astype(BF16_NP)
    in_maps = []
    for b in range(B):
        in_maps.append({
            "tokT": np.ascontiguousarray(tokens[b].T).astype(qdt),
            "ctxT": np.ascontiguousarray(context[b].T).astype(BF16_NP),
            "wqp": wqp, "wkp": wkp, "wv": wv_, "wo": wo_,
        })
    return in_maps


def kernel(tokens, context, Wq, Wk, Wv, Wo, bo):
    nc = _get_program()
    in_maps = _prepare_in_maps(tokens, context, Wq, Wk, Wv, Wo, bo)
    res = bass_utils.run_bass_kernel_spmd(nc, in_maps, core_ids=list(range(B)))
    return np.stack([res.results[b]["out"] for b in range(B)])
